# revision 57
# baseline (speedup 1.0000x reference)
import sys
import numpy as np
import ml_dtypes

sys.path.insert(0, "/opt/trn_rl_repo")

import concourse.bass as bass
import concourse.tile as tile
from concourse import mybir
from concourse.bass_utils import run_bass_kernel_spmd

F32 = mybir.dt.float32
F32R = mybir.dt.float32r
BF16 = mybir.dt.bfloat16
AF = mybir.ActivationFunctionType
ALU = mybir.AluOpType
AX = mybir.AxisListType

HID = 128
NT = 64        # tokens per core (half of 128)
NA = 1024      # atoms per core (all atoms)
NG = 64        # ligand graphs
NI = 4         # images
NCORES = 8

# engine for each in-loop leaky-relu chunk, indexed by token % 16:
# 'a' = ACT (scalar engine), 'v' = DVE (vector). (Pool/gpsimd cannot read
# PSUM, so the post-matmul leaky-relu can only run on ACT or DVE; Pool
# instead generates the per-token scaled weights Wj.)
LR_PATTERN = ['a', 'v'] * 32
LR_PATTERN[33] = 'a'
C1_RED = ['v', 'a', 'v', 'a', 'v', 'a', 'v']   # conv1 chunk-reduce engines
HBUFS = 5
WJBUFS = 8
GRP_T = 7

# f32 small-blob column indices
C_WPE, C_WPG, C_WB2 = 0, 1, 2
C_BTOK, C_BPK, C_BCAT, C_BGATEH = 3, 4, 5, 6
C_BATOM, C_BGRAPH, C_BB1, C_BINT, C_BPGH, C_BB2 = 7, 8, 9, 10, 11, 12
C_RPE, C_RPG = 13, 14
FW_COLS = 15

# bf16 weight-blob column offsets
O_WINT = 0            # [128, 128]
O_WTOK = 128          # 2 chunks of 128
O_WPK = 384           # 2 chunks
O_WCAT = 640          # 3 chunks
O_WGATE = 1024        # 3 chunks
O_WB1 = 1408          # 2 chunks
O_S = 1664            # one-hot segment matrix [128, 512]
BW_COLS = 2176

TRACE = False
TRACE_KW = {}
LAST = None


_COMPUTE_INSTS = (
    "InstActivation", "InstTensorCopy", "InstTensorScalar", "InstTensorScalarPtr",
    "InstTensorTensor", "InstTensorTensorReduce", "InstTensorReduce", "InstMemSet",
    "InstMatmult", "InstScalarTensorTensor", "InstTensorTensorScan", "InstLdweights",
    "InstDMACopy", "InstDMATransposeAnt", "InstTriggeredCopy", "InstDrain",
    "InstEventSemaphoreOp", "InstSemaphoreOp", "InstCopy", "InstIota", "InstSelect",
)


def _legalize_waits(nc):
    # walrus in this toolchain accepts at most ONE sync wait on TPB compute
    # instructions; hoist extras into same-engine NoOps placed just before.
    k = 0
    for f in nc.m.functions:
        for blk in f.blocks:
            insts = blk.instructions
            out = []
            for ins in insts:
                si = getattr(ins, "sync_info", None)
                if (si is not None and len(si.on_wait) > 1
                        and type(ins).__name__ in _COMPUTE_INSTS):
                    waits = list(si.on_wait)
                    for w in waits[:-1]:
                        nop = mybir.InstNoOp(
                            name=f"WNOP-{k}", engine=ins.engine,
                            sync_info=mybir.SyncInfo(on_wait=[w], on_update=[]))
                        k += 1
                        out.append(nop)
                    ins.sync_info = mybir.SyncInfo(on_wait=[waits[-1]],
                                                   on_update=list(si.on_update))
                out.append(ins)
            blk.instructions = out
    return k


def build_program(bpe: float, bpg: float, bb2: float, bint_zero: bool = True, sim_trace: bool = False) -> bass.Bass:
    nc = bass.Bass()

    # ---- DRAM inputs (per-core views; same names across SPMD cores) ----
    d_tw = nc.dram_tensor("TW", [128, 642], F32R, kind="ExternalInput")
    d_la = nc.dram_tensor("laT", [64, NA + 256], BF16, kind="ExternalInput")
    d_m0 = nc.dram_tensor("msf0", [96, 4096 + 9 * HID], BF16, kind="ExternalInput")
    d_m1 = nc.dram_tensor("msf1", [64, 512 + NG], BF16, kind="ExternalInput")
    d_bw = nc.dram_tensor("BW", [128, BW_COLS], BF16, kind="ExternalInput")
    d_fw = nc.dram_tensor("FW", [128, FW_COLS], F32, kind="ExternalInput")
    d_w0 = nc.dram_tensor("W0t", [64, 27 * HID], BF16, kind="ExternalInput")

    d_out = nc.dram_tensor("out", [1, 128], F32, kind="ExternalOutput")

    tc_ref = tile.TileContext(nc, trace_sim=sim_trace)
    with tc_ref as tc:
        with (
            # f32r is bit-identical to f32; accumulating into f32r tiles is
            # full precision — this only silences the dtype-name check.
            nc.allow_low_precision(reason="f32r accumulators are fp32-width"),
            tc.tile_pool(name="const", bufs=1) as cpool,
            tc.tile_pool(name="pre_sb", bufs=2) as prepool,
            tc.tile_pool(name="wj", bufs=WJBUFS) as wjpool,
            tc.tile_pool(name="h", bufs=HBUFS) as hpool,
            tc.tile_pool(name="gate", bufs=2) as gpool,
            tc.tile_pool(name="junk", bufs=2) as jpool,
            tc.tile_pool(name="ps_y", bufs=3, space="PSUM") as psy,
            tc.tile_pool(name="ps_misc", bufs=1, space="PSUM") as pspre,
        ):
            # ---------- DMA loads (order = SP dispatch order; hot first) ----------
            fw = cpool.tile([128, FW_COLS], F32, tag="fw")
            nc.sync.dma_start(fw[:], d_fw[:])
            tw = cpool.tile([128, 642], F32R, tag="tw")
            nc.sync.dma_start(tw[:], d_tw[:])
            tf = tw[:, 0:256]
            wtok32 = tw[:, 256:640]
            la = prepool.tile([64, NA + 256], BF16, tag="la")
            nc.sync.dma_start(la[:], d_la[:])
            m1 = prepool.tile([64, 512 + NG], BF16, tag="m1")
            nc.sync.dma_start(m1[:], d_m1[:])
            m0w = cpool.tile([96, 4096 + 9 * HID], BF16, tag="m0w")
            nc.sync.dma_start(m0w[:], d_m0w := d_m0[:])
            m0 = m0w[:, 0:4096]
            w96 = m0w[:, 4096:4096 + 9 * HID]
            w0 = cpool.tile([64, 27 * HID], BF16, tag="w0")
            nc.sync.dma_start(w0[:], d_w0[:])
            bw = cpool.tile([128, BW_COLS], BF16, tag="bw")
            nc.sync.dma_start(bw[:], d_bw[:])

            wint = tw[:, 512:640]
            wpeg = tw[:, 640:642]
            watom = la[0:64, NA:NA + 128]
            wgraph = la[0:64, NA + 128:NA + 256]
            bcol = lambda c: fw[:, c:c + 1]

            # ---------- ACT table warmup (overlaps the first DMAs) ----------
            warm = jpool.tile([1, 1], F32, tag="warm")
            nc.vector.memset(warm[:], 0.0)
            nc.scalar.activation(warm[:], warm[:], AF.Silu)

            # ---------- preamble: tok / atoms ----------
            # tfr = silu(tf) directly on ACT (Silu in same table as Prelu/Tanh)
            tfr = prepool.tile([128, 256], F32R, tag="tfr")
            nc.scalar.activation(tfr[:], tf, AF.Silu)
            ps_tok = pspre.tile([128, 128], F32, tag="ps")
            for q in range(2):
                nc.tensor.matmul(ps_tok[:], tw[:, 256 + 128 * q:256 + 128 * (q + 1)],
                                 tfr[:, 128 * q:128 * (q + 1)],
                                 start=(q == 0), stop=(q == 1))
            tokT = cpool.tile([128, 128], F32, tag="tokT")
            nc.scalar.activation(tokT[:], ps_tok[:], AF.Identity, bias=bcol(C_BTOK))

            ps_at = psy.tile([128, NA], F32, tag="y")
            for v in range(2):
                nc.tensor.matmul(ps_at[:, 512 * v:512 * (v + 1)], watom,
                                 la[:, 512 * v:512 * (v + 1)], start=True, stop=True)
            atomsT = cpool.tile([128, NA], F32R, tag="atomsT")
            nc.vector.tensor_scalar(atomsT[:], ps_at[:], bcol(C_BATOM), 0.0,
                                    op0=ALU.add, op1=ALU.bypass)

            # ---------- preamble tasks: fine-grained ops, dispensed between
            # main-loop tokens so no engine sees a long foreign burst ----------
            state = {}
            x3 = cpool.tile([96, 4096], BF16, tag="x3")
            x3v = x3[:, :].rearrange("p (z q) -> p z q", z=16)
            x3v = x3v.rearrange("p z (b d) -> p z b d", b=16)  # [96,16,16,16]
            p1parts = cpool.tile([128, 7], F32, tag="p1p")

            tok_tasks = []
            grp_tasks = {}

            # conv windows only read x in [0,14) of each 16-wide row; skip
            # the never-read tail columns in the silu (3584 of 4096 cols)
            m0v = m0.rearrange("p (z q) -> p z q", z=16)
            m0v = m0v.rearrange("p z (b d) -> p z b d", b=16)
            for q in range(4):
                tok_tasks.append(lambda q=q: nc.scalar.activation(
                    x3v[:, 4 * q:4 * (q + 1), :, 0:14],
                    m0v[:, 4 * q:4 * (q + 1), :, 0:14], AF.Silu))

            def t_x0():
                x0 = prepool.tile([64, 512], BF16, tag="x0")
                nc.scalar.activation(x0[:], m1[:, 0:512], AF.Silu)
                state["x0"] = x0
            tok_tasks.append(t_x0)

            def c1_chunk(c, eng):
                def run():
                    ps_c1 = pspre.tile([128, 392], F32, tag="ps")
                    out_ap = ps_c1[:, :].rearrange("p (a b c) -> p a b c", a=2, b=14)
                    for ti in range(9):
                        dz, dy = ti // 3, ti % 3
                        rhs = x3v[:, dz + 2 * c:dz + 2 * c + 2, dy:dy + 14, 0:14]
                        nc.tensor.matmul(out_ap, w96[:, ti * HID:(ti + 1) * HID],
                                         rhs, start=(ti == 0), stop=(ti == 8))
                    if eng == 'v':
                        nc.vector.tensor_reduce(p1parts[:, c:c + 1], ps_c1[:],
                                                axis=AX.X, op=ALU.add)
                    else:
                        junk = jpool.tile([128, 392], F32, tag="junkw")
                        nc.scalar.activation(junk[:], ps_c1[:], AF.Copy,
                                             accum_out=p1parts[:, c:c + 1])
                return run

            def t_conv0():
                x0 = state["x0"]
                x0v = x0[:, :].rearrange("p (z y x) -> p z y x", z=8, y=8)
                ps_c0 = pspre.tile([128, 216], F32, tag="ps")
                out0 = ps_c0[:, :].rearrange("p (a b c) -> p a b c", a=6, b=6)
                for ti in range(27):
                    dz, dy, dx = ti // 9, (ti // 3) % 3, ti % 3
                    rhs = x0v[:, dz:dz + 6, dy:dy + 6, dx:dx + 6]
                    nc.tensor.matmul(out0, w0[:, ti * HID:(ti + 1) * HID], rhs,
                                     start=(ti == 0), stop=(ti == 26))
                p0 = prepool.tile([128, 1], F32, tag="p0")
                nc.vector.tensor_reduce(p0[:], ps_c0[:], axis=AX.X, op=ALU.add)
                sp0 = prepool.tile([128, 1], BF16, tag="sp0")
                nc.scalar.activation(sp0[:], p0[:], AF.Silu, scale=1.0 / 216.0)
                state["sp0"] = sp0

            def t_p1():
                p1 = prepool.tile([128, 1], F32, tag="p1")
                nc.vector.tensor_reduce(p1[:], p1parts[:], axis=AX.X, op=ALU.add)
                sp1 = prepool.tile([128, 1], BF16, tag="sp1")
                nc.scalar.activation(sp1[:], p1[:], AF.Silu, scale=1.0 / 2744.0)
                state["sp1"] = sp1
            def t_pocket_pf():
                ps_pk = pspre.tile([128, 1], F32, tag="ps")
                nc.tensor.matmul(ps_pk[:], bw[:, O_WPK:O_WPK + 128], state["sp0"][:],
                                 start=True, stop=False)
                nc.tensor.matmul(ps_pk[:], bw[:, O_WPK + 128:O_WPK + 256], state["sp1"][:],
                                 start=False, stop=True)
                pocket = prepool.tile([128, 1], BF16, tag="pocket")
                nc.scalar.activation(pocket[:], ps_pk[:], AF.Identity, bias=bcol(C_BPK))
                tok_sum = prepool.tile([128, 1], BF16, tag="toksum")
                nc.vector.tensor_reduce(tok_sum[:], tokT[:], axis=AX.X, op=ALU.add)
                ps_pf = pspre.tile([128, 2], F32, tag="ps")
                chunks = [pocket, tok_sum, tok_sum]
                for q in range(3):
                    nc.tensor.matmul(ps_pf[:, 0:1], bw[:, O_WCAT + 128 * q:O_WCAT + 128 * (q + 1)],
                                     chunks[q][:], start=(q == 0), stop=(q == 2))
                for q in range(3):
                    nc.tensor.matmul(ps_pf[:, 1:2], bw[:, O_WGATE + 128 * q:O_WGATE + 128 * (q + 1)],
                                     chunks[q][:], start=(q == 0), stop=(q == 2))
                # W_cat/b_cat pre-scaled by 0.5 on host, so
                # pf = pflin_half * (tanh(0.5*gate + bpgh) + 1)
                pft = prepool.tile([128, 1], F32, tag="pft")
                nc.scalar.activation(pft[:], ps_pf[:, 1:2], AF.Tanh,
                                     bias=bcol(C_BGATEH), scale=0.5)
                pflin = prepool.tile([128, 1], F32, tag="pflin")
                nc.scalar.activation(pflin[:], ps_pf[:, 0:1], AF.Identity, bias=bcol(C_BCAT))
                pf = prepool.tile([128, 1], BF16, tag="pf")
                nc.vector.scalar_tensor_tensor(pf[:], pft[:], 1.0, pflin[:],
                                               op0=ALU.add, op1=ALU.mult)
                state["pf"] = pf

            def t_bias():
                pf = state["pf"]
                ps_gf = pspre.tile([128, NG], F32, tag="ps")
                nc.tensor.matmul(ps_gf[:], wgraph, m1[:, 512:512 + NG], start=True, stop=True)
                gfT = prepool.tile([128, NG], BF16, tag="gfT")
                nc.scalar.activation(gfT[:], ps_gf[:], AF.Identity, bias=bcol(C_BGRAPH))
                ps_u = pspre.tile([128, 1], F32, tag="ps")
                nc.tensor.matmul(ps_u[:], bw[:, O_WB1:O_WB1 + 128], pf[:],
                                 start=True, stop=True)
                ub = prepool.tile([128, 1], F32, tag="ub")
                nc.scalar.activation(ub[:], ps_u[:], AF.Identity, bias=bcol(C_BB1))
                ps_hb = pspre.tile([128, NG], F32, tag="ps")
                nc.tensor.matmul(ps_hb[:], bw[:, O_WB1 + 128:O_WB1 + 256], gfT[:],
                                 start=True, stop=True)
                hb = prepool.tile([128, NG], F32, tag="hb")
                nc.scalar.activation(hb[:], ps_hb[:], AF.Prelu, bias=ub[:], alpha=0.01)
                ps_b2 = pspre.tile([1, NG], F32, tag="ps")
                nc.tensor.matmul(ps_b2[:], fw[:, C_WB2:C_WB2 + 1], hb[:],
                                 start=True, stop=True)
                nc.scalar.activation(out_sb[:, 64:128], ps_b2[:], AF.Identity,
                                     bias=fw[0:1, C_BB2:C_BB2 + 1])


            grp_tasks[0] = [c1_chunk(0, C1_RED[0])]
            grp_tasks[1] = [c1_chunk(1, C1_RED[1])]
            grp_tasks[2] = [c1_chunk(2, C1_RED[2]), c1_chunk(3, C1_RED[3])]
            grp_tasks[3] = [c1_chunk(4, C1_RED[4]), c1_chunk(5, C1_RED[5])]
            grp_tasks[4] = [c1_chunk(6, C1_RED[6])]
            grp_tasks[5] = [t_conv0, t_p1]
            grp_tasks[6] = [t_pocket_pf, t_bias]
            tok_iter = iter(tok_tasks)

            # ---------- main loop: 64 tokens in 8 groups of 8 ----------
            # leaky-relu is decomposed exactly: lrelu(v) = 0.99*relu(v) + 0.01*v.
            # The z matmuls consume relu(v) against wpeg pre-scaled by 0.99 on
            # the host; the 0.01*v part is linear, so its pe/pg contribution
            # lin_r[i,j] = sum_k a[k,i] * (0.01*r_r[k]) * tok[k,j] (with
            # r_r = W_int @ W_pe|W_pg from host FW cols) is accumulated into
            # the same PSUM banks by tiny matmuls issued before the z matmuls.
            out_sb = prepool.tile([1, 128], F32, tag="outsb")
            aep = cpool.tile([128, 32], F32, tag="aep")

            tokbf = cpool.tile([128, NT], BF16, tag="tokbf")
            tokbf = cpool.tile([128, NT], BF16, tag="tokbf")
            nc.scalar.activation(tokbf[:], tokT[:, 0:NT], AF.Copy)
            ar = cpool.tile([128, 2 * NA], BF16, tag="ar")
            lin_sb = cpool.tile([128, NA], F32, tag="linsb")
            linv = lin_sb[:, :].rearrange("p (a r j) -> p a r j", a=8, r=2)

            def t_ar(r):
                def run():
                    nc.gpsimd.tensor_scalar_mul(ar[:, NA * r:NA * (r + 1)], atomsT[:],
                                                bcol(C_RPE if r == 0 else C_RPG))
                return run

            def t_tokbf():
                nc.scalar.activation(tokbf[:], tokT[:, 0:NT], AF.Copy)

            def t_linmm():
                # lin_ps[p, 128*a + 64*r + j] = lin_r[128a+p, j]
                lin_ps = psy.tile([128, NA], F32, tag="y")
                for a in range(8):
                    for r in range(2):
                        nc.tensor.matmul(lin_ps[:, 128 * a + 64 * r:128 * a + 64 * r + 64],
                                         ar[:, NA * r + 128 * a:NA * r + 128 * (a + 1)],
                                         tokbf[:], start=True, stop=True)
                state["lin_ps"] = lin_ps

            def t_lincopy():
                nc.scalar.activation(lin_sb[:], state["lin_ps"][:], AF.Copy)

            # final order: x3q0, ar0, ar1, linmm, lincopy, x3q1..q3, x0
            tok_tasks[1:1] = [t_ar(0), t_tokbf, t_ar(1), t_linmm, t_lincopy]

            zb = None
            for g in range(8):
                if g % 2 == 0:
                    zb = pspre.tile([128, 256], F32, tag="z")
                for t in range(8):
                    j = 8 * g + t
                    eng = LR_PATTERN[j]
                    wj = wjpool.tile([128, 128], F32R, tag="wj")
                    nc.gpsimd.tensor_scalar_mul(wj[:], wint, tokT[:, j:j + 1])
                    y = psy.tile([128, NA], F32, tag="y")
                    for v in range(2):
                        nc.tensor.matmul(y[:, 512 * v:512 * (v + 1)], wj[:],
                                         atomsT[:, 512 * v:512 * (v + 1)],
                                         start=True, stop=True)
                    h = hpool.tile([128, NA], F32R, tag="h")
                    if eng == 'a':
                        nc.scalar.activation(h[:], y[:], AF.Relu,
                                             bias=bcol(C_BINT))
                    else:
                        nc.vector.tensor_scalar(h[:], y[:], bcol(C_BINT), 0.0,
                                                op0=ALU.add, op1=ALU.max)
                    # zb col = 32*a + 2*(8*(g%2) + t) + r
                    for a in range(8):
                        col = 32 * a + 2 * (8 * (g % 2) + t)
                        nc.tensor.matmul(zb[:, col:col + 2],
                                         h[:, 128 * a:128 * (a + 1)], wpeg[:],
                                         start=True, stop=True)
                    if j >= 3 and t % 2 == 1:
                        fn = next(tok_iter, None)
                        if fn is not None:
                            fn()
                    if t == GRP_T:
                        for fn in grp_tasks.get(g - 1, []):
                            fn()
                if g % 2 == 1:
                    b = g // 2
                    # zc_r = zb_r + lin_r (+ bpe on the pe plane); zb carries the
                    # 0.99 factor via host-scaled wpeg
                    zbv = zb[:, :].rearrange("p (a jj r) -> p a jj r", a=8, jj=16)
                    zc = gpool.tile([128, 256], F32, tag="zc")
                    zcv = zc[:, :].rearrange("p (r a jj) -> p r a jj", r=2, a=8)
                    for r in range(2):
                        lap = linv[:, :, r, 16 * b:16 * (b + 1)]
                        nc.vector.scalar_tensor_tensor(
                            zcv[:, r], zbv[:, :, :, r], bpe if r == 0 else 0.0,
                            lap, op0=ALU.add, op1=ALU.add)
                    s = gpool.tile([128, 128], F32, tag="s")
                    nc.scalar.activation(s[:], zc[:, 128:256], AF.Tanh,
                                         bias=bcol(C_BPGH), scale=0.5)
                    w = gpool.tile([128, 128], F32, tag="w")
                    nc.gpsimd.tensor_scalar(w[:], s[:], 0.5, 0.5,
                                            op0=ALU.mult, op1=ALU.add)
                    t_ = gpool.tile([128, 128], F32, tag="t")
                    nc.vector.tensor_mul(t_[:], zc[:, 0:128], w[:])
                    tv = t_[:, :].rearrange("p (a jj) -> p a jj", a=8)
                    nc.vector.tensor_reduce(aep[:, 8 * b:8 * b + 8], tv,
                                            axis=AX.X, op=ALU.add)

            # ---------- tail: atom energies -> segments ----------
            ae8 = prepool.tile([128, 8], BF16, tag="ae8")
            nc.vector.tensor_reduce(ae8[:], aep[:, :].rearrange("p (b a) -> p a b", b=4),
                                    axis=AX.X, op=ALU.add)
            ps_seg = pspre.tile([1, NG], F32, tag="ps")
            for a in range(8):
                nc.tensor.matmul(ps_seg[:], ae8[:, a:a + 1], bw[:, O_S + 64 * a:O_S + 64 * (a + 1)],
                                 start=(a == 0), stop=(a == 7))
            nc.vector.tensor_scalar(out_sb[:, 0:64], ps_seg[:], 0.0, 0.0,
                                    op0=ALU.add, op1=ALU.bypass)
            nc.sync.dma_start(d_out[:], out_sb[:])

    _legalize_waits(nc)
    nc._tile_ctx = tc_ref
    return nc


def kernel(**inputs) -> np.ndarray:
    f = lambda a: np.ascontiguousarray(np.asarray(a), dtype=np.float32)
    bf = lambda a: np.ascontiguousarray(np.asarray(a, dtype=np.float32)).astype(ml_dtypes.bfloat16)
    tf = f(inputs["token_features"])
    la = f(inputs["lig_atom"])
    lg = f(inputs["lig_graph"])
    m0 = f(inputs["ms_feat_0"])
    m1 = f(inputs["ms_feat_1"])
    lb = np.asarray(inputs["ligand_batch"])

    # one-hot segment matrix, atom-chunk-major: S[p, 64q+s] = [batch[128q+p]==s]
    S = (lb[:, None] == np.arange(NG)[None, :]).astype(np.float32)  # [1024, 64]
    Sh = np.zeros((128, 512), np.float32)
    for q in range(8):
        Sh[:, 64 * q:64 * (q + 1)] = S[128 * q:128 * (q + 1)]

    Wc1 = f(inputs["Wc1"])  # [128, 32, 3,3,3]
    Wc0 = f(inputs["Wc0"])  # [128, 64, 3,3,3]
    # W96[32*dx+c, 128*(3*dz+dy)+o] = Wc1[o,c,dz,dy,dx]
    W96 = Wc1.transpose(2, 3, 4, 1, 0).reshape(9, 96, HID)
    W96 = np.ascontiguousarray(W96.transpose(1, 0, 2).reshape(96, 9 * HID))
    W0t = Wc0.transpose(2, 3, 4, 1, 0).reshape(27, 64, HID)
    W0t = np.ascontiguousarray(W0t.transpose(1, 0, 2).reshape(64, 27 * HID))

    wcat = 0.5 * f(inputs["W_cat"])
    wgate = f(inputs["W_gate"]).copy()
    wcat[2 * HID:] /= 128.0   # token mean = sum / 128
    wgate[2 * HID:] /= 128.0

    # bf16 weight blob [128, BW_COLS]
    BW = np.zeros((128, BW_COLS), np.float32)
    BW[:, O_WINT:O_WINT + 128] = f(inputs["W_int"])
    BW[:, O_WTOK:O_WTOK + 256] = f(inputs["W_token"]).reshape(2, 128, HID).transpose(1, 0, 2).reshape(128, 256)
    BW[:, O_WPK:O_WPK + 256] = f(inputs["W_pocket"]).reshape(2, 128, HID).transpose(1, 0, 2).reshape(128, 256)
    BW[:, O_WCAT:O_WCAT + 384] = wcat.reshape(3, 128, HID).transpose(1, 0, 2).reshape(128, 384)
    BW[:, O_WGATE:O_WGATE + 384] = wgate.reshape(3, 128, HID).transpose(1, 0, 2).reshape(128, 384)
    BW[:, O_WB1:O_WB1 + 256] = f(inputs["W_bias1"]).reshape(2, 128, HID).transpose(1, 0, 2).reshape(128, 256)
    WAG = np.zeros((64, 256), np.float32)
    WAG[:, 0:128] = f(inputs["W_atom"])
    WAG[:, 128:256] = f(inputs["W_graph"])
    WAGb = WAG.astype(ml_dtypes.bfloat16)
    BW[:, O_S:O_S + 512] = Sh

    # f32 small blob [128, FW_COLS]
    col = lambda a: f(a).reshape(-1)
    FW = np.zeros((128, FW_COLS), np.float32)
    FW[:, C_WPE] = col(inputs["W_pe"])
    FW[:, C_WPG] = col(inputs["W_pg"])
    FW[:, C_WB2] = col(inputs["W_bias2"])
    FW[:, C_BTOK] = col(inputs["b_token"])
    FW[:, C_BPK] = col(inputs["b_pocket"])
    FW[:, C_BCAT] = 0.5 * col(inputs["b_cat"])
    FW[:, C_BGATEH] = col(inputs["b_gate"]) * 0.5
    FW[:, C_BATOM] = col(inputs["b_atom"])
    FW[:, C_BGRAPH] = col(inputs["b_graph"])
    FW[:, C_BB1] = col(inputs["b_bias1"])
    FW[:, C_BINT] = col(inputs["b_int"])
    wpe_v = f(inputs["W_pe"]).reshape(-1)
    wpg_v = f(inputs["W_pg"]).reshape(-1)
    wint_f = f(inputs["W_int"])
    bint_v = col(inputs["b_int"])
    FW[:, C_RPE] = 0.01 * (wint_f @ wpe_v)
    FW[:, C_RPG] = 0.01 * (wint_f @ wpg_v)
    bpe_eff = float(np.asarray(inputs["b_pe"]).reshape(-1)[0]) + 0.01 * float(wpe_v @ bint_v)
    bpg_eff = float(np.asarray(inputs["b_pg"]).reshape(-1)[0]) + 0.01 * float(wpg_v @ bint_v)
    FW[:, C_BPGH] = bpg_eff * 0.5
    FW[:, C_BB2] = float(np.asarray(inputs["b_bias2"]).reshape(-1)[0])

    Wpeg = 0.99 * np.concatenate([f(inputs["W_pe"]).reshape(128, 1),
                                  f(inputs["W_pg"]).reshape(128, 1)], axis=1)

    W96b = W96.astype(ml_dtypes.bfloat16)
    bpe = bpe_eff
    bpg = bpg_eff
    bb2 = float(np.asarray(inputs["b_bias2"]).reshape(-1)[0])

    TW_shared = np.concatenate([
        f(inputs["W_token"]).reshape(2, 128, HID).transpose(1, 0, 2).reshape(128, 256),
        f(inputs["W_int"]), Wpeg.astype(np.float32)], axis=1)
    shared = {
        "BW": BW.astype(ml_dtypes.bfloat16),
        "FW": FW,

        "W0t": W0t.astype(ml_dtypes.bfloat16),

    }

    in_maps = []
    for c in range(NCORES):
        n, h = c // 2, c % 2
        m = dict(shared)
        # permute tokens: this core's 64 first
        perm = np.concatenate([np.arange(64 * h, 64 * (h + 1)),
                               np.arange(64 * (1 - h), 64 * (2 - h))])
        tfp = tf[n][perm]                       # [128 tok, 256 feat]
        # dram is [128, 256] = 2 chunks of features stacked on cols
        m["TW"] = np.concatenate([
            np.ascontiguousarray(tfp.T.reshape(2, 128, 128).transpose(1, 0, 2).reshape(128, 256)),
            TW_shared], axis=1)
        m["laT"] = np.concatenate([bf(la[n].T), WAGb], axis=1)  # [64, 1024+256]
        m["lgT"] = bf(lg[n].T)                  # [64, 64]
        m0f = m0[n].reshape(32, 4096)
        x3h = np.zeros((96, 4096), dtype=np.float32)
        for dd in range(3):
            x3h[32 * dd:32 * (dd + 1), 0:4096 - dd] = m0f[:, dd:]
        m["msf0"] = np.concatenate([bf(x3h), W96b], axis=1)
        m["msf1"] = np.concatenate([bf(m1[n].reshape(64, 512)), bf(lg[n].T)], axis=1)
        in_maps.append(m)

    bint_zero = bool(np.all(np.asarray(inputs['b_int']) == 0.0))
    nc = build_program(bpe, bpg, bb2, bint_zero)
    r = run_bass_kernel_spmd(nc, in_maps, core_ids=list(range(NCORES)),
                             trace=TRACE, **(TRACE_KW if TRACE else {}))
    global LAST
    LAST = r
    res = r.results

    out = np.zeros((NI, NG), dtype=np.float32)
    for n in range(NI):
        out[n] = (res[2 * n]["out"][0, 0:64] + res[2 * n + 1]["out"][0, 0:64]
                  + res[2 * n]["out"][0, 64:128])
    return out


# revision 58
# speedup vs baseline: 1.0125x; 1.0125x over previous
import sys
import numpy as np
import ml_dtypes

sys.path.insert(0, "/opt/trn_rl_repo")

import concourse.bass as bass
import concourse.tile as tile
from concourse import mybir
from concourse.bass_utils import run_bass_kernel_spmd

F32 = mybir.dt.float32
F32R = mybir.dt.float32r
BF16 = mybir.dt.bfloat16
AF = mybir.ActivationFunctionType
ALU = mybir.AluOpType
AX = mybir.AxisListType

HID = 128
NT = 64        # tokens per core (half of 128)
NA = 1024      # atoms per core (all atoms)
NG = 64        # ligand graphs
NI = 4         # images
NCORES = 8

# engine for each in-loop leaky-relu chunk, indexed by token % 16:
# 'a' = ACT (scalar engine), 'v' = DVE (vector). (Pool/gpsimd cannot read
# PSUM, so the post-matmul leaky-relu can only run on ACT or DVE; Pool
# instead generates the per-token scaled weights Wj.)
LR_PATTERN = ['a', 'v'] * 32
LR_PATTERN[33] = 'a'
C1_RED = ['v', 'a', 'v', 'a', 'v', 'a', 'v']   # conv1 chunk-reduce engines
HBUFS = 5
WJBUFS = 8
GRP_T = 7

# f32 small-blob column indices
C_WPE, C_WPG, C_WB2 = 0, 1, 2
C_BTOK, C_BPK, C_BCAT, C_BGATEH = 3, 4, 5, 6
C_BATOM, C_BGRAPH, C_BB1, C_BINT, C_BPGH, C_BB2 = 7, 8, 9, 10, 11, 12
C_RPE, C_RPG = 13, 14
FW_COLS = 15

# bf16 weight-blob column offsets
O_WINT = 0            # [128, 128]
O_WTOK = 128          # 2 chunks of 128
O_WPK = 384           # 2 chunks
O_WCAT = 640          # 3 chunks
O_WGATE = 1024        # 3 chunks
O_WB1 = 1408          # 2 chunks
O_S = 1664            # one-hot segment matrix [128, 512]
BW_COLS = 2176

TRACE = False
TRACE_KW = {}
LAST = None


_COMPUTE_INSTS = (
    "InstActivation", "InstTensorCopy", "InstTensorScalar", "InstTensorScalarPtr",
    "InstTensorTensor", "InstTensorTensorReduce", "InstTensorReduce", "InstMemSet",
    "InstMatmult", "InstScalarTensorTensor", "InstTensorTensorScan", "InstLdweights",
    "InstDMACopy", "InstDMATransposeAnt", "InstTriggeredCopy", "InstDrain",
    "InstEventSemaphoreOp", "InstSemaphoreOp", "InstCopy", "InstIota", "InstSelect",
)


def _legalize_waits(nc):
    # walrus in this toolchain accepts at most ONE sync wait on TPB compute
    # instructions; hoist extras into same-engine NoOps placed just before.
    k = 0
    for f in nc.m.functions:
        for blk in f.blocks:
            insts = blk.instructions
            out = []
            for ins in insts:
                si = getattr(ins, "sync_info", None)
                if (si is not None and len(si.on_wait) > 1
                        and type(ins).__name__ in _COMPUTE_INSTS):
                    waits = list(si.on_wait)
                    for w in waits[:-1]:
                        nop = mybir.InstNoOp(
                            name=f"WNOP-{k}", engine=ins.engine,
                            sync_info=mybir.SyncInfo(on_wait=[w], on_update=[]))
                        k += 1
                        out.append(nop)
                    ins.sync_info = mybir.SyncInfo(on_wait=[waits[-1]],
                                                   on_update=list(si.on_update))
                out.append(ins)
            blk.instructions = out
    return k


def build_program(bpe: float, bpg: float, bb2: float, bint_zero: bool = True, sim_trace: bool = False) -> bass.Bass:
    nc = bass.Bass()

    # ---- DRAM inputs (per-core views; same names across SPMD cores) ----
    d_tf = nc.dram_tensor("tfT", [128, 256], F32, kind="ExternalInput")
    d_wtok32 = nc.dram_tensor("Wtok32", [128, 384], F32R, kind="ExternalInput")
    d_la = nc.dram_tensor("laT", [64, NA + 256], BF16, kind="ExternalInput")
    d_m0 = nc.dram_tensor("msf0", [96, 4096], BF16, kind="ExternalInput")
    d_m1 = nc.dram_tensor("msf1", [64, 512 + NG], BF16, kind="ExternalInput")
    d_bw = nc.dram_tensor("BW", [128, BW_COLS], BF16, kind="ExternalInput")
    d_fw = nc.dram_tensor("FW", [128, FW_COLS], F32, kind="ExternalInput")
    d_wpeg = nc.dram_tensor("Wpeg", [128, 2], F32R, kind="ExternalInput")
    d_w96 = nc.dram_tensor("W96", [96, 9 * HID], BF16, kind="ExternalInput")
    d_w0 = nc.dram_tensor("W0t", [64, 27 * HID], BF16, kind="ExternalInput")

    d_out = nc.dram_tensor("out", [1, 128], F32, kind="ExternalOutput")

    tc_ref = tile.TileContext(nc, trace_sim=sim_trace)
    with tc_ref as tc:
        with (
            # f32r is bit-identical to f32; accumulating into f32r tiles is
            # full precision — this only silences the dtype-name check.
            nc.allow_low_precision(reason="f32r accumulators are fp32-width"),
            tc.tile_pool(name="const", bufs=1) as cpool,
            tc.tile_pool(name="pre_sb", bufs=2) as prepool,
            tc.tile_pool(name="wj", bufs=WJBUFS) as wjpool,
            tc.tile_pool(name="h", bufs=HBUFS) as hpool,
            tc.tile_pool(name="gate", bufs=2) as gpool,
            tc.tile_pool(name="junk", bufs=2) as jpool,
            tc.tile_pool(name="ps_y", bufs=3, space="PSUM") as psy,
            tc.tile_pool(name="ps_misc", bufs=1, space="PSUM") as pspre,
        ):
            # ---------- DMA loads (order = SP dispatch order; hot first) ----------
            fw = cpool.tile([128, FW_COLS], F32, tag="fw")
            nc.sync.dma_start(fw[:], d_fw[:])
            tf = prepool.tile([128, 256], F32, tag="tf")
            nc.sync.dma_start(tf[:], d_tf[:])
            wtok32 = cpool.tile([128, 384], F32R, tag="wtok32")
            nc.sync.dma_start(wtok32[:], d_wtok32[:])
            la = prepool.tile([64, NA + 256], BF16, tag="la")
            nc.sync.dma_start(la[:], d_la[:])
            wpeg = cpool.tile([128, 2], F32R, tag="wpeg")
            nc.sync.dma_start(wpeg[:], d_wpeg[:])
            m1 = prepool.tile([64, 512 + NG], BF16, tag="m1")
            nc.sync.dma_start(m1[:], d_m1[:])
            m0 = cpool.tile([96, 4096], BF16, tag="m0")
            nc.sync.dma_start(m0[:], d_m0[:])
            w96 = cpool.tile([96, 9 * HID], BF16, tag="w96")
            nc.sync.dma_start(w96[:], d_w96[:])
            w0 = cpool.tile([64, 27 * HID], BF16, tag="w0")
            nc.sync.dma_start(w0[:], d_w0[:])
            bw = cpool.tile([128, BW_COLS], BF16, tag="bw")
            nc.sync.dma_start(bw[:], d_bw[:])

            wint = wtok32[:, 256:384]
            watom = la[0:64, NA:NA + 128]
            wgraph = la[0:64, NA + 128:NA + 256]
            bcol = lambda c: fw[:, c:c + 1]

            # ---------- ACT table warmup (overlaps the first DMAs) ----------
            warm = jpool.tile([1, 1], F32, tag="warm")
            nc.vector.memset(warm[:], 0.0)
            nc.scalar.activation(warm[:], warm[:], AF.Silu)

            # ---------- preamble: tok / atoms ----------
            # tfr = silu(tf) directly on ACT (Silu in same table as Prelu/Tanh)
            tfr = prepool.tile([128, 256], F32R, tag="tfr")
            nc.scalar.activation(tfr[:], tf[:], AF.Silu)
            ps_tok = pspre.tile([128, 128], F32, tag="ps")
            for q in range(2):
                nc.tensor.matmul(ps_tok[:], wtok32[:, 128 * q:128 * (q + 1)],
                                 tfr[:, 128 * q:128 * (q + 1)],
                                 start=(q == 0), stop=(q == 1))
            tokT = cpool.tile([128, 128], F32, tag="tokT")
            nc.scalar.activation(tokT[:], ps_tok[:], AF.Identity, bias=bcol(C_BTOK))

            ps_at = psy.tile([128, NA], F32, tag="y")
            for v in range(2):
                nc.tensor.matmul(ps_at[:, 512 * v:512 * (v + 1)], watom,
                                 la[:, 512 * v:512 * (v + 1)], start=True, stop=True)
            atomsT = cpool.tile([128, NA], F32R, tag="atomsT")
            nc.vector.tensor_scalar(atomsT[:], ps_at[:], bcol(C_BATOM), 0.0,
                                    op0=ALU.add, op1=ALU.bypass)

            # ---------- preamble tasks: fine-grained ops, dispensed between
            # main-loop tokens so no engine sees a long foreign burst ----------
            state = {}
            x3 = cpool.tile([96, 4096], BF16, tag="x3")
            x3v = x3[:, :].rearrange("p (z q) -> p z q", z=16)
            x3v = x3v.rearrange("p z (b d) -> p z b d", b=16)  # [96,16,16,16]
            p1parts = cpool.tile([128, 7], F32, tag="p1p")

            tok_tasks = []
            grp_tasks = {}

            # conv windows only read x in [0,14) of each 16-wide row; skip
            # the never-read tail columns in the silu (3584 of 4096 cols)
            m0v = m0[:, :].rearrange("p (z q) -> p z q", z=16)
            m0v = m0v.rearrange("p z (b d) -> p z b d", b=16)
            for q in range(4):
                tok_tasks.append(lambda q=q: nc.scalar.activation(
                    x3v[:, 4 * q:4 * (q + 1), :, 0:14],
                    m0v[:, 4 * q:4 * (q + 1), :, 0:14], AF.Silu))

            def t_x0():
                x0 = prepool.tile([64, 512], BF16, tag="x0")
                nc.scalar.activation(x0[:], m1[:, 0:512], AF.Silu)
                state["x0"] = x0
            tok_tasks.append(t_x0)

            def c1_chunk(c, eng):
                def run():
                    ps_c1 = pspre.tile([128, 392], F32, tag="ps")
                    out_ap = ps_c1[:, :].rearrange("p (a b c) -> p a b c", a=2, b=14)
                    for ti in range(9):
                        dz, dy = ti // 3, ti % 3
                        rhs = x3v[:, dz + 2 * c:dz + 2 * c + 2, dy:dy + 14, 0:14]
                        nc.tensor.matmul(out_ap, w96[:, ti * HID:(ti + 1) * HID],
                                         rhs, start=(ti == 0), stop=(ti == 8))
                    if eng == 'v':
                        nc.vector.tensor_reduce(p1parts[:, c:c + 1], ps_c1[:],
                                                axis=AX.X, op=ALU.add)
                    else:
                        junk = jpool.tile([128, 392], F32, tag="junkw")
                        nc.scalar.activation(junk[:], ps_c1[:], AF.Copy,
                                             accum_out=p1parts[:, c:c + 1])
                return run

            def t_conv0():
                x0 = state["x0"]
                x0v = x0[:, :].rearrange("p (z y x) -> p z y x", z=8, y=8)
                ps_c0 = pspre.tile([128, 216], F32, tag="ps")
                out0 = ps_c0[:, :].rearrange("p (a b c) -> p a b c", a=6, b=6)
                for ti in range(27):
                    dz, dy, dx = ti // 9, (ti // 3) % 3, ti % 3
                    rhs = x0v[:, dz:dz + 6, dy:dy + 6, dx:dx + 6]
                    nc.tensor.matmul(out0, w0[:, ti * HID:(ti + 1) * HID], rhs,
                                     start=(ti == 0), stop=(ti == 26))
                p0 = prepool.tile([128, 1], F32, tag="p0")
                nc.vector.tensor_reduce(p0[:], ps_c0[:], axis=AX.X, op=ALU.add)
                sp0 = prepool.tile([128, 1], BF16, tag="sp0")
                nc.scalar.activation(sp0[:], p0[:], AF.Silu, scale=1.0 / 216.0)
                state["sp0"] = sp0

            def t_p1():
                p1 = prepool.tile([128, 1], F32, tag="p1")
                nc.vector.tensor_reduce(p1[:], p1parts[:], axis=AX.X, op=ALU.add)
                sp1 = prepool.tile([128, 1], BF16, tag="sp1")
                nc.scalar.activation(sp1[:], p1[:], AF.Silu, scale=1.0 / 2744.0)
                state["sp1"] = sp1
            def t_pocket_pf():
                ps_pk = pspre.tile([128, 1], F32, tag="ps")
                nc.tensor.matmul(ps_pk[:], bw[:, O_WPK:O_WPK + 128], state["sp0"][:],
                                 start=True, stop=False)
                nc.tensor.matmul(ps_pk[:], bw[:, O_WPK + 128:O_WPK + 256], state["sp1"][:],
                                 start=False, stop=True)
                pocket = prepool.tile([128, 1], BF16, tag="pocket")
                nc.scalar.activation(pocket[:], ps_pk[:], AF.Identity, bias=bcol(C_BPK))
                tok_sum = prepool.tile([128, 1], BF16, tag="toksum")
                nc.vector.tensor_reduce(tok_sum[:], tokT[:], axis=AX.X, op=ALU.add)
                ps_pf = pspre.tile([128, 2], F32, tag="ps")
                chunks = [pocket, tok_sum, tok_sum]
                for q in range(3):
                    nc.tensor.matmul(ps_pf[:, 0:1], bw[:, O_WCAT + 128 * q:O_WCAT + 128 * (q + 1)],
                                     chunks[q][:], start=(q == 0), stop=(q == 2))
                for q in range(3):
                    nc.tensor.matmul(ps_pf[:, 1:2], bw[:, O_WGATE + 128 * q:O_WGATE + 128 * (q + 1)],
                                     chunks[q][:], start=(q == 0), stop=(q == 2))
                # W_cat/b_cat pre-scaled by 0.5 on host, so
                # pf = pflin_half * (tanh(0.5*gate + bpgh) + 1)
                pft = prepool.tile([128, 1], F32, tag="pft")
                nc.scalar.activation(pft[:], ps_pf[:, 1:2], AF.Tanh,
                                     bias=bcol(C_BGATEH), scale=0.5)
                pflin = prepool.tile([128, 1], F32, tag="pflin")
                nc.scalar.activation(pflin[:], ps_pf[:, 0:1], AF.Identity, bias=bcol(C_BCAT))
                pf = prepool.tile([128, 1], BF16, tag="pf")
                nc.vector.scalar_tensor_tensor(pf[:], pft[:], 1.0, pflin[:],
                                               op0=ALU.add, op1=ALU.mult)
                state["pf"] = pf

            def t_bias():
                pf = state["pf"]
                ps_gf = pspre.tile([128, NG], F32, tag="ps")
                nc.tensor.matmul(ps_gf[:], wgraph, m1[:, 512:512 + NG], start=True, stop=True)
                gfT = prepool.tile([128, NG], BF16, tag="gfT")
                nc.scalar.activation(gfT[:], ps_gf[:], AF.Identity, bias=bcol(C_BGRAPH))
                ps_u = pspre.tile([128, 1], F32, tag="ps")
                nc.tensor.matmul(ps_u[:], bw[:, O_WB1:O_WB1 + 128], pf[:],
                                 start=True, stop=True)
                ub = prepool.tile([128, 1], F32, tag="ub")
                nc.scalar.activation(ub[:], ps_u[:], AF.Identity, bias=bcol(C_BB1))
                ps_hb = pspre.tile([128, NG], F32, tag="ps")
                nc.tensor.matmul(ps_hb[:], bw[:, O_WB1 + 128:O_WB1 + 256], gfT[:],
                                 start=True, stop=True)
                hb = prepool.tile([128, NG], F32, tag="hb")
                nc.scalar.activation(hb[:], ps_hb[:], AF.Prelu, bias=ub[:], alpha=0.01)
                ps_b2 = pspre.tile([1, NG], F32, tag="ps")
                nc.tensor.matmul(ps_b2[:], fw[:, C_WB2:C_WB2 + 1], hb[:],
                                 start=True, stop=True)
                nc.scalar.activation(out_sb[:, 64:128], ps_b2[:], AF.Identity,
                                     bias=fw[0:1, C_BB2:C_BB2 + 1])


            grp_tasks[0] = [c1_chunk(0, C1_RED[0])]
            grp_tasks[1] = [c1_chunk(1, C1_RED[1])]
            grp_tasks[2] = [c1_chunk(2, C1_RED[2]), c1_chunk(3, C1_RED[3])]
            grp_tasks[3] = [c1_chunk(4, C1_RED[4]), c1_chunk(5, C1_RED[5])]
            grp_tasks[4] = [c1_chunk(6, C1_RED[6])]
            grp_tasks[5] = [t_conv0, t_p1]
            grp_tasks[6] = [t_pocket_pf, t_bias]
            tok_iter = iter(tok_tasks)

            # ---------- main loop: 64 tokens in 8 groups of 8 ----------
            # leaky-relu is decomposed exactly: lrelu(v) = 0.99*relu(v) + 0.01*v.
            # The z matmuls consume relu(v) against wpeg pre-scaled by 0.99 on
            # the host; the 0.01*v part is linear, so its pe/pg contribution
            # lin_r[i,j] = sum_k a[k,i] * (0.01*r_r[k]) * tok[k,j] (with
            # r_r = W_int @ W_pe|W_pg from host FW cols) is accumulated into
            # the same PSUM banks by tiny matmuls issued before the z matmuls.
            out_sb = prepool.tile([1, 128], F32, tag="outsb")
            aep = cpool.tile([128, 32], F32, tag="aep")

            tokbf = cpool.tile([128, NT], BF16, tag="tokbf")
            tokbf = cpool.tile([128, NT], BF16, tag="tokbf")
            nc.scalar.activation(tokbf[:], tokT[:, 0:NT], AF.Copy)
            ar = cpool.tile([128, 2 * NA], BF16, tag="ar")
            lin_sb = cpool.tile([128, NA], F32, tag="linsb")
            linv = lin_sb[:, :].rearrange("p (a r j) -> p a r j", a=8, r=2)

            def t_ar(r):
                def run():
                    nc.gpsimd.tensor_scalar_mul(ar[:, NA * r:NA * (r + 1)], atomsT[:],
                                                bcol(C_RPE if r == 0 else C_RPG))
                return run

            def t_tokbf():
                nc.scalar.activation(tokbf[:], tokT[:, 0:NT], AF.Copy)

            def t_linmm():
                # lin_ps[p, 128*a + 64*r + j] = lin_r[128a+p, j]
                lin_ps = psy.tile([128, NA], F32, tag="y")
                for a in range(8):
                    for r in range(2):
                        nc.tensor.matmul(lin_ps[:, 128 * a + 64 * r:128 * a + 64 * r + 64],
                                         ar[:, NA * r + 128 * a:NA * r + 128 * (a + 1)],
                                         tokbf[:], start=True, stop=True)
                state["lin_ps"] = lin_ps

            def t_lincopy():
                nc.scalar.activation(lin_sb[:], state["lin_ps"][:], AF.Copy)

            # final order: x3q0, ar0, ar1, linmm, lincopy, x3q1..q3, x0
            tok_tasks[1:1] = [t_ar(0), t_tokbf, t_ar(1), t_linmm, t_lincopy]

            zb = None
            for g in range(8):
                if g % 2 == 0:
                    zb = pspre.tile([128, 256], F32, tag="z")
                for t in range(8):
                    j = 8 * g + t
                    eng = LR_PATTERN[j]
                    wj = wjpool.tile([128, 128], F32R, tag="wj")
                    nc.gpsimd.tensor_scalar_mul(wj[:], wint, tokT[:, j:j + 1])
                    y = psy.tile([128, NA], F32, tag="y")
                    for v in range(2):
                        nc.tensor.matmul(y[:, 512 * v:512 * (v + 1)], wj[:],
                                         atomsT[:, 512 * v:512 * (v + 1)],
                                         start=True, stop=True)
                    h = hpool.tile([128, NA], F32R, tag="h")
                    if eng == 'a':
                        nc.scalar.activation(h[:], y[:], AF.Relu,
                                             bias=bcol(C_BINT))
                    else:
                        nc.vector.tensor_scalar(h[:], y[:], bcol(C_BINT), 0.0,
                                                op0=ALU.add, op1=ALU.max)
                    # zb col = 32*a + 2*(8*(g%2) + t) + r
                    for a in range(8):
                        col = 32 * a + 2 * (8 * (g % 2) + t)
                        nc.tensor.matmul(zb[:, col:col + 2],
                                         h[:, 128 * a:128 * (a + 1)], wpeg[:],
                                         start=True, stop=True)
                    if j >= 3 and t % 2 == 1:
                        fn = next(tok_iter, None)
                        if fn is not None:
                            fn()
                    if t == GRP_T:
                        for fn in grp_tasks.get(g - 1, []):
                            fn()
                if g % 2 == 1:
                    b = g // 2
                    # zc_r = zb_r + lin_r (+ bpe on the pe plane); zb carries the
                    # 0.99 factor via host-scaled wpeg
                    zbv = zb[:, :].rearrange("p (a jj r) -> p a jj r", a=8, jj=16)
                    zc = gpool.tile([128, 256], F32, tag="zc")
                    zcv = zc[:, :].rearrange("p (r a jj) -> p r a jj", r=2, a=8)
                    for r in range(2):
                        lap = linv[:, :, r, 16 * b:16 * (b + 1)]
                        nc.vector.scalar_tensor_tensor(
                            zcv[:, r], zbv[:, :, :, r], bpe if r == 0 else 0.0,
                            lap, op0=ALU.add, op1=ALU.add)
                    s = gpool.tile([128, 128], F32, tag="s")
                    nc.scalar.activation(s[:], zc[:, 128:256], AF.Tanh,
                                         bias=bcol(C_BPGH), scale=0.5)
                    w = gpool.tile([128, 128], F32, tag="w")
                    nc.gpsimd.tensor_scalar(w[:], s[:], 0.5, 0.5,
                                            op0=ALU.mult, op1=ALU.add)
                    t_ = gpool.tile([128, 128], F32, tag="t")
                    nc.vector.tensor_mul(t_[:], zc[:, 0:128], w[:])
                    tv = t_[:, :].rearrange("p (a jj) -> p a jj", a=8)
                    nc.vector.tensor_reduce(aep[:, 8 * b:8 * b + 8], tv,
                                            axis=AX.X, op=ALU.add)

            # ---------- tail: atom energies -> segments ----------
            ae8 = prepool.tile([128, 8], BF16, tag="ae8")
            nc.vector.tensor_reduce(ae8[:], aep[:, :].rearrange("p (b a) -> p a b", b=4),
                                    axis=AX.X, op=ALU.add)
            ps_seg = pspre.tile([1, NG], F32, tag="ps")
            for a in range(8):
                nc.tensor.matmul(ps_seg[:], ae8[:, a:a + 1], bw[:, O_S + 64 * a:O_S + 64 * (a + 1)],
                                 start=(a == 0), stop=(a == 7))
            nc.vector.tensor_scalar(out_sb[:, 0:64], ps_seg[:], 0.0, 0.0,
                                    op0=ALU.add, op1=ALU.bypass)
            nc.sync.dma_start(d_out[:], out_sb[:])

    _legalize_waits(nc)
    nc._tile_ctx = tc_ref
    return nc


def kernel(**inputs) -> np.ndarray:
    f = lambda a: np.ascontiguousarray(np.asarray(a), dtype=np.float32)
    bf = lambda a: np.ascontiguousarray(np.asarray(a, dtype=np.float32)).astype(ml_dtypes.bfloat16)
    tf = f(inputs["token_features"])
    la = f(inputs["lig_atom"])
    lg = f(inputs["lig_graph"])
    m0 = f(inputs["ms_feat_0"])
    m1 = f(inputs["ms_feat_1"])
    lb = np.asarray(inputs["ligand_batch"])

    # one-hot segment matrix, atom-chunk-major: S[p, 64q+s] = [batch[128q+p]==s]
    S = (lb[:, None] == np.arange(NG)[None, :]).astype(np.float32)  # [1024, 64]
    Sh = np.zeros((128, 512), np.float32)
    for q in range(8):
        Sh[:, 64 * q:64 * (q + 1)] = S[128 * q:128 * (q + 1)]

    Wc1 = f(inputs["Wc1"])  # [128, 32, 3,3,3]
    Wc0 = f(inputs["Wc0"])  # [128, 64, 3,3,3]
    # W96[32*dx+c, 128*(3*dz+dy)+o] = Wc1[o,c,dz,dy,dx]
    W96 = Wc1.transpose(2, 3, 4, 1, 0).reshape(9, 96, HID)
    W96 = np.ascontiguousarray(W96.transpose(1, 0, 2).reshape(96, 9 * HID))
    W0t = Wc0.transpose(2, 3, 4, 1, 0).reshape(27, 64, HID)
    W0t = np.ascontiguousarray(W0t.transpose(1, 0, 2).reshape(64, 27 * HID))

    wcat = 0.5 * f(inputs["W_cat"])
    wgate = f(inputs["W_gate"]).copy()
    wcat[2 * HID:] /= 128.0   # token mean = sum / 128
    wgate[2 * HID:] /= 128.0

    # bf16 weight blob [128, BW_COLS]
    BW = np.zeros((128, BW_COLS), np.float32)
    BW[:, O_WINT:O_WINT + 128] = f(inputs["W_int"])
    BW[:, O_WTOK:O_WTOK + 256] = f(inputs["W_token"]).reshape(2, 128, HID).transpose(1, 0, 2).reshape(128, 256)
    BW[:, O_WPK:O_WPK + 256] = f(inputs["W_pocket"]).reshape(2, 128, HID).transpose(1, 0, 2).reshape(128, 256)
    BW[:, O_WCAT:O_WCAT + 384] = wcat.reshape(3, 128, HID).transpose(1, 0, 2).reshape(128, 384)
    BW[:, O_WGATE:O_WGATE + 384] = wgate.reshape(3, 128, HID).transpose(1, 0, 2).reshape(128, 384)
    BW[:, O_WB1:O_WB1 + 256] = f(inputs["W_bias1"]).reshape(2, 128, HID).transpose(1, 0, 2).reshape(128, 256)
    WAG = np.zeros((64, 256), np.float32)
    WAG[:, 0:128] = f(inputs["W_atom"])
    WAG[:, 128:256] = f(inputs["W_graph"])
    WAGb = WAG.astype(ml_dtypes.bfloat16)
    BW[:, O_S:O_S + 512] = Sh

    # f32 small blob [128, FW_COLS]
    col = lambda a: f(a).reshape(-1)
    FW = np.zeros((128, FW_COLS), np.float32)
    FW[:, C_WPE] = col(inputs["W_pe"])
    FW[:, C_WPG] = col(inputs["W_pg"])
    FW[:, C_WB2] = col(inputs["W_bias2"])
    FW[:, C_BTOK] = col(inputs["b_token"])
    FW[:, C_BPK] = col(inputs["b_pocket"])
    FW[:, C_BCAT] = 0.5 * col(inputs["b_cat"])
    FW[:, C_BGATEH] = col(inputs["b_gate"]) * 0.5
    FW[:, C_BATOM] = col(inputs["b_atom"])
    FW[:, C_BGRAPH] = col(inputs["b_graph"])
    FW[:, C_BB1] = col(inputs["b_bias1"])
    FW[:, C_BINT] = col(inputs["b_int"])
    wpe_v = f(inputs["W_pe"]).reshape(-1)
    wpg_v = f(inputs["W_pg"]).reshape(-1)
    wint_f = f(inputs["W_int"])
    bint_v = col(inputs["b_int"])
    FW[:, C_RPE] = 0.01 * (wint_f @ wpe_v)
    FW[:, C_RPG] = 0.01 * (wint_f @ wpg_v)
    bpe_eff = float(np.asarray(inputs["b_pe"]).reshape(-1)[0]) + 0.01 * float(wpe_v @ bint_v)
    bpg_eff = float(np.asarray(inputs["b_pg"]).reshape(-1)[0]) + 0.01 * float(wpg_v @ bint_v)
    FW[:, C_BPGH] = bpg_eff * 0.5
    FW[:, C_BB2] = float(np.asarray(inputs["b_bias2"]).reshape(-1)[0])

    Wpeg = 0.99 * np.concatenate([f(inputs["W_pe"]).reshape(128, 1),
                                  f(inputs["W_pg"]).reshape(128, 1)], axis=1)

    bpe = bpe_eff
    bpg = bpg_eff
    bb2 = float(np.asarray(inputs["b_bias2"]).reshape(-1)[0])

    shared = {
        "BW": BW.astype(ml_dtypes.bfloat16),
        "FW": FW,
        "Wpeg": Wpeg,
        "W96": W96.astype(ml_dtypes.bfloat16),
        "W0t": W0t.astype(ml_dtypes.bfloat16),
        "Wtok32": np.concatenate([
            f(inputs["W_token"]).reshape(2, 128, HID).transpose(1, 0, 2).reshape(128, 256),
            f(inputs["W_int"])], axis=1).copy(),
    }

    in_maps = []
    for c in range(NCORES):
        n, h = c // 2, c % 2
        m = dict(shared)
        # permute tokens: this core's 64 first
        perm = np.concatenate([np.arange(64 * h, 64 * (h + 1)),
                               np.arange(64 * (1 - h), 64 * (2 - h))])
        tfp = tf[n][perm]                       # [128 tok, 256 feat]
        # dram is [128, 256] = 2 chunks of features stacked on cols
        m["tfT"] = np.ascontiguousarray(tfp.T.reshape(2, 128, 128).transpose(1, 0, 2).reshape(128, 256))
        m["laT"] = np.concatenate([bf(la[n].T), WAGb], axis=1)  # [64, 1024+256]
        m["lgT"] = bf(lg[n].T)                  # [64, 64]
        m0f = m0[n].reshape(32, 4096)
        x3h = np.zeros((96, 4096), dtype=np.float32)
        for dd in range(3):
            x3h[32 * dd:32 * (dd + 1), 0:4096 - dd] = m0f[:, dd:]
        m["msf0"] = bf(x3h)
        m["msf1"] = np.concatenate([bf(m1[n].reshape(64, 512)), bf(lg[n].T)], axis=1)
        in_maps.append(m)

    bint_zero = bool(np.all(np.asarray(inputs['b_int']) == 0.0))
    nc = build_program(bpe, bpg, bb2, bint_zero)
    r = run_bass_kernel_spmd(nc, in_maps, core_ids=list(range(NCORES)),
                             trace=TRACE, **(TRACE_KW if TRACE else {}))
    global LAST
    LAST = r
    res = r.results

    out = np.zeros((NI, NG), dtype=np.float32)
    for n in range(NI):
        out[n] = (res[2 * n]["out"][0, 0:64] + res[2 * n + 1]["out"][0, 0:64]
                  + res[2 * n]["out"][0, 64:128])
    return out
